# revision 1
# baseline (speedup 1.0000x reference)
"""Trainium2 Bass kernel for nn_Conv2dTB (BN -> ternary quantize -> 3x3 conv
-> beta box-filter scaling), data-parallel over batch on 8 NeuronCores.

Contract: kernel(**inputs) takes the FULL unsharded inputs as numpy arrays and
returns the FULL [16, 256, 56, 56] float32 output. Internally the batch dim is
split 2 images/core; BN batch statistics use an on-device AllReduce so the
normalization matches the reference's full-batch statistics.
"""

import numpy as np

# Problem shapes (hardcoded per contract).
N, C, H, W = 16, 256, 56, 56
COUT = 256
KS = 3
EPS = 1e-4
N_CORES = 8
NLOC = N // N_CORES  # images per core (2)
CB = C // 128  # channel blocks (2)
COB = COUT // 128  # cout blocks (2)
RT_ROWS = 8  # image rows per pixel tile
NT = H // RT_ROWS  # row tiles per image (7)
NPIX = RT_ROWS * W  # pixels per tile (448)
HW = H * W  # 3136
PH = H + 2  # padded rows (58)
PW = W + 2  # padded cols (58)
COUNT = float(N * H * W)  # BN reduction count (full batch)

_CACHE = {}


def _build():
    import concourse.tile as tile
    from concourse import bacc, mybir

    f32 = mybir.dt.float32
    f16 = mybir.dt.float16
    AF = mybir.ActivationFunctionType
    ALU = mybir.AluOpType

    nc = bacc.Bacc("TRN2", target_bir_lowering=False, debug=False,
                   num_devices=N_CORES)

    # ---- external I/O ----
    x_d = nc.dram_tensor("x", [NLOC, C, H, W], f32, kind="ExternalInput").ap()
    gamma_d = nc.dram_tensor("bn_gamma", [C], f32, kind="ExternalInput").ap()
    bnbeta_d = nc.dram_tensor("bn_beta", [C], f32, kind="ExternalInput").ap()
    w_d = nc.dram_tensor("conv_w", [COUT, C, KS, KS], f32,
                         kind="ExternalInput").ap()
    cb_d = nc.dram_tensor("conv_b", [COUT], f32, kind="ExternalInput").ap()
    bb_d = nc.dram_tensor("beta_conv_b", [1], f32, kind="ExternalInput").ap()
    # host-provided constants
    ident_d = nc.dram_tensor("ident128", [128, 128], f32,
                             kind="ExternalInput").ap()
    ident56_d = nc.dram_tensor("ident56", [H, H], f32,
                               kind="ExternalInput").ap()
    t3_d = nc.dram_tensor("tridiag", [H, H], f32, kind="ExternalInput").ap()
    cnt_d = nc.dram_tensor("boxcnt", [H, W], f32, kind="ExternalInput").ap()
    out_d = nc.dram_tensor("out", [NLOC, COUT, H, W], f32,
                           kind="ExternalOutput").ap()

    with tile.TileContext(nc) as tc:
        with (
            tc.tile_pool(name="persist", bufs=1) as persist,
            tc.tile_pool(name="scratch", bufs=3) as scratch,
            tc.tile_pool(name="stage", bufs=3) as stage,
            tc.tile_pool(name="outp", bufs=6) as outp,
            tc.tile_pool(name="betabc", bufs=4) as betabc,
            tc.tile_pool(name="ps_y", bufs=5, space="PSUM") as ps_y,
            tc.tile_pool(name="ps_c", bufs=1, space="PSUM") as ps_c,
            tc.tile_pool(name="ps_m", bufs=2, space="PSUM") as ps_m,
            tc.tile_pool(name="dram", bufs=1, space="DRAM") as dram,
        ):
            # ---------------- load inputs ----------------
            # DMA contiguous runs must stay <= 8KB/partition -> split loads
            x_sb = persist.tile([128, NLOC, CB, HW], f32)
            for cb in range(CB):
                for img in range(NLOC):
                    xv = x_d[img].rearrange("(cb p) h w -> cb p (h w)", p=128)
                    for a in range(2):
                        sl = slice(a * (HW // 2), (a + 1) * (HW // 2))
                        nc.sync.dma_start(out=x_sb[:, img, cb, sl],
                                          in_=xv[cb][:, sl])

            w_sb = persist.tile([128, COB, C, KS * KS], f32)
            wv = w_d.rearrange("(cob p) c k1 k2 -> cob p c (k1 k2)", p=128)
            for cob in range(COB):
                for a in range(2):
                    sl = slice(a * (C // 2), (a + 1) * (C // 2))
                    nc.gpsimd.dma_start(out=w_sb[:, cob, sl, :],
                                        in_=wv[cob][:, sl, :])

            ident_sb = persist.tile([128, 128], f32)
            nc.gpsimd.dma_start(out=ident_sb[:], in_=ident_d[:])
            ident56_sb = persist.tile([H, H], f32)
            nc.gpsimd.dma_start(out=ident56_sb[:], in_=ident56_d[:])
            t3_sb = persist.tile([H, H], f32)
            nc.gpsimd.dma_start(out=t3_sb[:], in_=t3_d[:])
            cnt_sb = persist.tile([H, W], f32)
            nc.gpsimd.dma_start(out=cnt_sb[:], in_=cnt_d[:])

            gamma_sb = persist.tile([128, CB], f32)
            nc.gpsimd.dma_start(out=gamma_sb[:],
                              in_=gamma_d.rearrange("(cb p) -> p cb", p=128))
            bnbeta_sb = persist.tile([128, CB], f32)
            nc.gpsimd.dma_start(out=bnbeta_sb[:],
                              in_=bnbeta_d.rearrange("(cb p) -> p cb", p=128))

            convb_cols = persist.tile([128, COB], f32)
            nc.gpsimd.dma_start(out=convb_cols[:],
                              in_=cb_d.rearrange("(cob p) -> p cob", p=128))

            import concourse.bass as bass

            bb56 = persist.tile([H, 1], f32)
            bbsrc = bb_d[0:1]
            nc.sync.dma_start(
                out=bb56[:],
                in_=bass.AP(tensor=bbsrc.tensor, offset=bbsrc.offset,
                            ap=[[0, H], [1, 1]]),
            )

            # ones for channel-sum contraction
            ones_c = persist.tile([128, 1], f16)
            nc.vector.memset(ones_c[:], 1.0)

            # 1 / (256 * boxcount + beta_conv_b)
            den56 = persist.tile([H, W], f32)
            nc.vector.tensor_scalar(den56[:], cnt_sb[:], 256.0, bb56[:],
                                    ALU.mult, ALU.add)
            invden = persist.tile([H, W], f32)
            nc.vector.reciprocal(invden[:], den56[:])

            # ---------------- weight prep: [cout, c] -> bf16 [c, cout] ------
            w_bf = persist.tile([128, COB, C, KS * KS], f16)
            for cob in range(COB):
                nc.scalar.copy(w_bf[:, cob, :, :], w_sb[:, cob, :, :])
            ident_bf = persist.tile([128, 128], f16)
            nc.vector.tensor_copy(ident_bf[:], ident_sb[:])
            wT = persist.tile([128, CB, KS * KS, COB, 128], f16)
            for cob in range(COB):
                for cbk in range(CB):
                    for tap in range(KS * KS):
                        wsl = w_bf[:, cob, cbk * 128:(cbk + 1) * 128, tap]
                        ps_t = ps_m.tile([128, 128], f16, tag="psm")
                        nc.tensor.transpose(ps_t[:], wsl, ident_bf[:])
                        nc.scalar.copy(wT[:, cbk, tap, cob, :], ps_t[:])

            # ---------------- BN statistics + AllReduce ----------------
            # chunked to pipeline with the x DMA stream
            stats = persist.tile([128, CB, NLOC, 2, 2], f32)
            partial = persist.tile([128, 2 * CB], f32)
            allred = persist.tile([128, 2 * CB], f32)
            scale = persist.tile([128, CB], f32)
            shift = persist.tile([128, CB], f32)
            for cb in range(CB):
                for img in range(NLOC):
                    for a in range(2):
                        sl = slice(a * (HW // 2), (a + 1) * (HW // 2))
                        nc.vector.reduce_sum(stats[:, cb, img, a, 0:1],
                                             x_sb[:, img, cb, sl],
                                             axis=mybir.AxisListType.X)
                        sq = scratch.tile([128, HW // 2], f32, tag="sq")
                        nc.scalar.activation(
                            sq[:], x_sb[:, img, cb, sl], AF.Square,
                            accum_out=stats[:, cb, img, a, 1:2])
                    nc.vector.tensor_add(stats[:, cb, img, 0, :],
                                         stats[:, cb, img, 0, :],
                                         stats[:, cb, img, 1, :])
                nc.vector.tensor_add(partial[:, 2 * cb:2 * cb + 2],
                                     stats[:, cb, 0, 0, :],
                                     stats[:, cb, 1, 0, :])

            bounce_in = dram.tile([128, 2 * CB], f32)
            bounce_out = dram.tile([128, 2 * CB], f32)
            nc.sync.dma_start(out=bounce_in[:], in_=partial[:])
            nc.gpsimd.collective_compute(
                "AllReduce", mybir.AluOpType.add,
                replica_groups=[list(range(N_CORES))],
                ins=[bounce_in.opt()], outs=[bounce_out.opt()],
            )
            nc.sync.dma_start(out=allred[:], in_=bounce_out[:])

            # scale/shift per cb: xn = x*scale + shift
            for cb in range(CB):
                mean = stage.tile([128, 1], f32, tag="mean")
                nc.vector.tensor_scalar_mul(mean[:], allred[:, 2 * cb:2 * cb + 1],
                                            1.0 / COUNT)
                var = stage.tile([128, 1], f32, tag="var")
                nc.vector.tensor_mul(var[:], mean[:], mean[:])
                ex2 = stage.tile([128, 1], f32, tag="ex2")
                nc.vector.tensor_scalar_mul(ex2[:],
                                            allred[:, 2 * cb + 1:2 * cb + 2],
                                            1.0 / COUNT)
                nc.vector.tensor_sub(var[:], ex2[:], var[:])
                nc.vector.tensor_scalar_add(var[:], var[:], EPS)
                rvar = stage.tile([128, 1], f32, tag="rvar")
                nc.vector.reciprocal(rvar[:], var[:])
                rstd = stage.tile([128, 1], f32, tag="rstd")
                nc.scalar.sqrt(rstd[:], rvar[:])
                nc.vector.tensor_mul(scale[:, cb:cb + 1], rstd[:],
                                     gamma_sb[:, cb:cb + 1])
                nc.vector.tensor_mul(shift[:, cb:cb + 1], mean[:],
                                     scale[:, cb:cb + 1])
                nc.vector.tensor_sub(shift[:, cb:cb + 1],
                                     bnbeta_sb[:, cb:cb + 1],
                                     shift[:, cb:cb + 1])

            # ---------------- ternarize + clip ----------------
            # t = sign(x*scale + shift) into zero-padded [PH, PW] buffer
            t_pad = persist.tile([128, CB, NLOC, PH, PW], f16)
            for cb in range(CB):
                for img in range(NLOC):
                    nc.vector.memset(t_pad[:, cb, img, 0, :], 0.0)
                    nc.vector.memset(t_pad[:, cb, img, PH - 1, :], 0.0)
                    nc.vector.memset(t_pad[:, cb, img, 1:PH - 1, 0], 0.0)
                    nc.vector.memset(t_pad[:, cb, img, 1:PH - 1, PW - 1], 0.0)

            # Signs first (they gate the conv), img0 first, in half-slab
            # chunks so the first conv tiles unblock as early as possible.
            xh = H // 2
            for img in range(NLOC):
                for half in range(2):
                    rs = slice(half * xh, (half + 1) * xh)
                    prs = slice(1 + half * xh, 1 + (half + 1) * xh)
                    for cb in range(CB):
                        tv = t_pad[:, cb, img, prs, 1:PW - 1]
                        nc.scalar.activation(
                            tv,
                            x_sb[:, img, cb, :].rearrange(
                                "p (h w) -> p h w", w=W)[:, rs, :],
                            AF.Sign, bias=shift[:, cb:cb + 1],
                            scale=scale[:, cb:cb + 1])

            xc_sb = persist.tile([128, CB, NLOC, HW], f16)
            c2_sb = persist.tile([128, NLOC, HW], f16)
            for img in range(NLOC):
                for cb in range(CB):
                    nc.scalar.activation(xc_sb[:, cb, img, :],
                                         x_sb[:, img, cb, :], AF.Abs,
                                         bias=shift[:, cb:cb + 1],
                                         scale=scale[:, cb:cb + 1])
                    nc.vector.tensor_scalar_min(xc_sb[:, cb, img, :],
                                                xc_sb[:, cb, img, :], 1.0)
                # channel-block pre-sum for the beta path (halves the PE
                # channel-sum matmuls)
                nc.vector.tensor_add(c2_sb[:, img, :], xc_sb[:, 0, img, :],
                                     xc_sb[:, 1, img, :])

            # ---------------- beta map (channel sums -> box filter) --------
            # cT_grid[x, 1+y] = sum_c min(|xn|,1)[c, y, x], built one image
            # row per matmul with the xc row-slice as the stationary operand
            # (channel contraction lands row pixels on partitions).
            bflat_ds = [dram.tile([H, W], f32, tag=f"bflat{i}",
                                  name=f"bflat{i}")
                        for i in range(NLOC)]
            cT_grid = persist.tile([H, NLOC, PW], f32)
            for img in range(NLOC):
                nc.vector.memset(cT_grid[:, img, 0:1], 0.0)
                nc.vector.memset(cT_grid[:, img, PW - 1:PW], 0.0)

            for img in range(NLOC):
                for rt in range(NT):
                    pct = ps_c.tile([H, RT_ROWS], f32)
                    for r in range(RT_ROWS):
                        y = rt * RT_ROWS + r
                        nc.tensor.matmul(
                            pct[:, r:r + 1],
                            c2_sb[:, img, y * W:(y + 1) * W],
                            ones_c[:], start=True, stop=True)
                    nc.scalar.copy(
                        cT_grid[:, img, 1 + rt * RT_ROWS:1 + (rt + 1) * RT_ROWS],
                        pct[:])

            for img in range(NLOC):
                # box over y (free dim), then over x via tridiagonal matmul
                hsumT = stage.tile([H, W], f32, tag="hsumT")
                cg = cT_grid[:, img, :]
                nc.vector.tensor_add(hsumT[:], cg[:, 0:W], cg[:, 1:W + 1])
                nc.vector.tensor_add(hsumT[:], hsumT[:], cg[:, 2:W + 2])
                pbT = ps_m.tile([H, W], f32, tag="psm")
                nc.tensor.matmul(pbT[:], t3_sb[:], hsumT[:], start=True,
                                 stop=True)
                bmapT = stage.tile([H, W], f32, tag="bmapT")
                nc.vector.tensor_scalar_add(bmapT[:], pbT[:], bb56[:])
                nc.vector.tensor_mul(bmapT[:], bmapT[:], invden[:])
                # transpose to [y, x] row-major and stage to DRAM
                pbm = ps_m.tile([H, W], f32, tag="psm")
                nc.tensor.transpose(pbm[:], bmapT[:], ident56_sb[:])
                bmap = stage.tile([H, W], f32, tag="bmap")
                nc.scalar.copy(bmap[:], pbm[:])
                nc.sync.dma_start(out=bflat_ds[img][:], in_=bmap[:])

            # ---------------- main conv + scale + store ----------------
            ov = out_d.rearrange("n (cob p) h w -> n cob p (h w)", p=128)
            for img in range(NLOC):
                for rt in range(NT):
                    bsl = bflat_ds[img][rt * RT_ROWS:(rt + 1) * RT_ROWS, :]
                    bbc = betabc.tile([128, NPIX], f32, tag="bbc")
                    nc.sync.dma_start(
                        out=bbc[:],
                        in_=bass.AP(tensor=bsl.tensor, offset=bsl.offset,
                                    ap=[[0, 128], [1, NPIX]]))
                    for cob in range(COB):
                        py = ps_y.tile([128, NPIX], f32)
                        first = True
                        for cbk in range(CB):
                            for ky in range(KS):
                                for kx in range(KS):
                                    rhs = t_pad[:, cbk, img,
                                                rt * RT_ROWS + ky:
                                                rt * RT_ROWS + ky + RT_ROWS,
                                                kx:kx + W]
                                    last = (cbk == CB - 1 and ky == KS - 1
                                            and kx == KS - 1)
                                    nc.tensor.matmul(
                                        py[:],
                                        wT[:, cbk, ky * KS + kx, cob, :],
                                        rhs, start=first, stop=last)
                                    first = False
                        # out = (conv + bias) * beta in one pass
                        osb = outp.tile([128, NPIX], f32, tag="osb")
                        nc.vector.scalar_tensor_tensor(
                            osb[:], py[:], convb_cols[:, cob:cob + 1], bbc[:],
                            ALU.add, ALU.mult)
                        nc.sync.dma_start(
                            out=ov[img, cob][:, rt * NPIX:(rt + 1) * NPIX],
                            in_=osb[:])

    nc.compile()
    return nc


def _consts():
    ident = np.eye(128, dtype=np.float32)
    ident56 = np.eye(H, dtype=np.float32)
    t3 = np.zeros((H, H), dtype=np.float32)
    for i in range(H):
        for j in range(max(0, i - 1), min(H, i + 2)):
            t3[j, i] = 1.0
    r = np.minimum(np.arange(H), H - 1 - np.arange(H))
    edge = (r >= 1).astype(np.float32) + 2.0  # 2 on border rows, 3 inside
    cnt = np.outer(edge, edge).astype(np.float32)  # valid taps: 4/6/9
    return ident, ident56, t3, cnt


def kernel(**inputs):
    from concourse.bass_utils import run_bass_kernel_spmd

    if "nc" not in _CACHE:
        _CACHE["nc"] = _build()
    nc = _CACHE["nc"]

    x = np.ascontiguousarray(inputs["x"], dtype=np.float32)
    ident, ident56, t3, cnt = _consts()
    shared = {
        "bn_gamma": np.ascontiguousarray(inputs["bn_gamma"], np.float32),
        "bn_beta": np.ascontiguousarray(inputs["bn_beta"], np.float32),
        "conv_w": np.ascontiguousarray(inputs["conv_w"], np.float32),
        "conv_b": np.ascontiguousarray(inputs["conv_b"], np.float32),
        "beta_conv_b": np.ascontiguousarray(inputs["beta_conv_b"], np.float32),
        "ident128": ident, "ident56": ident56, "tridiag": t3, "boxcnt": cnt,
    }
    in_maps = [
        {"x": np.ascontiguousarray(x[i * NLOC:(i + 1) * NLOC]), **shared}
        for i in range(N_CORES)
    ]
    res = run_bass_kernel_spmd(nc, in_maps, list(range(N_CORES)))
    out = np.concatenate([res.results[i]["out"] for i in range(N_CORES)],
                         axis=0)
    return out.astype(np.float32)

